# revision 28
# baseline (speedup 1.0000x reference)
"""Causal single-head attention (B=4, S=2048, D=768) on 8 TRN2 NeuronCores.

Sharding: core (b, h) = batch b, query-interleave h. Each core computes the
attention output for query tiles {2k+h : k=0..7} (128 rows each) of one
batch. Keys are fed ROTATED by 128*h so every core sees the identical score
structure: query tile k sits at rotated row 256k and attends rotated key
tiles 0..2k (tile 2k triangular) plus the wrap tile 15, which holds the
original first 128 keys for h=1 and is killed via the exp bias for h=0.

QK merge: scores = x (W_q^T W_k) x^T, with M = W_q^T W_k computed on the
host. The kernel computes TT = M^T xq^T (one projection instead of Q and K)
and uses the resident x tiles directly as the score stationaries. The TT
moving operand is read straight out of the xk tile (queries sit at even
rotated tiles), so no separate xq tensor is shipped.

xk layout [P, half(2), d(6), c(4), q(2), 128] matches HBM exactly (all DMAs
contiguous at line rate): even-j (query) tiles in half 0, d-major, shipped
first, so the TT phase (d-outer, et-inner) starts after ~160KB of DMA and
tracks arrival order. Non-TT-critical inputs (x2 second half, wv) are gated
behind the TT qi=0 sweep via WAW-dependency writes so they cannot steal
SDMA bandwidth from the critical stream.

All matmul inputs are bf16; accumulation stays f32 in PSUM; the output is
shipped bf16 and upcast on the host. Softmax denominators come from two
ones-columns appended to V (cols 768..769). Dummy matmuls at program start
keep the PE busy through the initial DMA wait so the HAM clock gate flips
to 2.4 GHz once, early, and never re-throttles.
"""

import os
import numpy as np
import ml_dtypes

import concourse.bass as bass
import concourse.mybir as mybir
import concourse.tile as tile
from concourse import bacc
from concourse.bass_utils import run_bass_kernel_spmd

B, S, D = 4, 2048, 768
P = 128
ND = D // P          # 6 contraction tiles
NQT = 8              # query tiles per core (128 rows each)
H = NQT * P          # 1024 query rows per core
NK = S // P          # 16 key tiles
SCALE = 1.0 / float(np.sqrt(D))
NEG = -10000.0
F32 = mybir.dt.float32
BF16 = mybir.dt.bfloat16
BF = ml_dtypes.bfloat16
N_WARMUP = 30        # dummy matmuls to warm the HAM clock gate (N=128 each)

_cached = {}
last_results = None

# jm position of tile j within its c-block: even j's first
_JMPOS = {0: 0, 2: 1, 1: 2, 3: 3}


def _k0(j):
    # first query tile whose score group includes key tile j (j < 15)
    return (j + 1) // 2


def _scores_phase(nc, tc, fb, xk, tts, ptp, dgp, pss):
    def xst(j):
        # stationary for key tile j: [128 d-rows, 128 key cols]
        return lambda dp: xk[:, j & 1, dp, j // 4, (j % 4) // 2, :]

    pts = {}
    for j in [15] + list(range(15)):
        k0 = 0 if j == 15 else _k0(j)
        ncol = (NQT - k0) * P
        pt = ptp.tile([P, H], BF16, name=f"pt{j}", tag="pt")
        pts[j] = pt
        st_j = xst(j)
        for qoff in range(0, ncol, 512):
            qw = min(512, ncol - qoff)
            st = pss.tile([P, 512], F32, tag="st")
            for dp in range(ND):
                nc.tensor.matmul(
                    st[:, :qw],
                    st_j(dp),
                    tts[dp][:, k0 * P + qoff:k0 * P + qoff + qw],
                    start=(dp == 0), stop=(dp == ND - 1),
                )
            if j == 15:
                nc.scalar.activation(
                    pt[:, qoff:qoff + qw], st[:, :qw],
                    mybir.ActivationFunctionType.Exp,
                    bias=fb[:, 0:1], scale=SCALE,
                )
            elif qoff == 0 and j % 2 == 0:
                # even j: leading tile is the diagonal triangle; odd j's
                # leading tile is already strictly below the diagonal
                mw = min(256, ncol)
                dg = dgp.tile([P, 256], F32, tag="dg")
                nc.vector.tensor_copy(dg[:, :mw], st[:, 0:mw])
                # keep where 256*k0 + 256*kk + f - (128*j + p) >= 0
                nc.gpsimd.affine_select(
                    out=dg[:, :mw], in_=dg[:, :mw],
                    compare_op=mybir.AluOpType.is_ge,
                    fill=NEG, base=256 * k0 - P * j,
                    pattern=([[256, 2], [1, P]] if mw == 256 else [[1, P]]),
                    channel_multiplier=-1,
                )
                nc.scalar.activation(
                    pt[:, 0:mw], dg[:, :mw],
                    mybir.ActivationFunctionType.Exp,
                    bias=0.0, scale=SCALE,
                )
                if qw > mw:
                    nc.scalar.activation(
                        pt[:, mw:qw], st[:, mw:qw],
                        mybir.ActivationFunctionType.Exp,
                        bias=0.0, scale=SCALE,
                    )
            else:
                nc.scalar.activation(
                    pt[:, qoff:qoff + qw], st[:, :qw],
                    mybir.ActivationFunctionType.Exp,
                    bias=0.0, scale=SCALE,
                )
    return pts


def _build_nc():
    nc = bacc.Bacc("TRN2", target_bir_lowering=False)

    # m d-major: col = 768*d + 128*et + c2  ->  M[128d+p, 128et+c2]
    m_d = nc.dram_tensor("m", [P, ND * D], BF16, kind="ExternalInput")
    # x keys: even-j half then odd-j half, d-major inside each:
    # even: col = 1024*d + 256*c + 128*q + i  (q=0,1 -> j = 4c, 4c+2)
    # odd:  col = 6144 + 1024*d + 256*c + 128*q + i  (j = 4c+1, 4c+3)
    xk_d = nc.dram_tensor("xk", [P, ND * S], BF16, kind="ExternalInput")
    # x keys again in [j, d] row layout + two ones columns, per-tile blocks
    x2_d = nc.dram_tensor("x2", [P, NK * (D + 2)], BF16, kind="ExternalInput")
    wvT_d = nc.dram_tensor("wvT", [P, ND * D], BF16, kind="ExternalInput")
    fb_d = nc.dram_tensor("fbias", [P, 1], F32, kind="ExternalInput")
    out_d = nc.dram_tensor("out", [H, D], BF16, kind="ExternalOutput")

    HLF = ND * S // 2    # 6144: column offset of the odd-j half in xk_d

    with tile.TileContext(nc) as tc:
        with (
            tc.tile_pool(name="cst", bufs=1) as cst,
            tc.tile_pool(name="wup", bufs=1) as wup,
            tc.tile_pool(name="xp", bufs=1) as xp,
            tc.tile_pool(name="x2p", bufs=1) as x2p,
            tc.tile_pool(name="wvp", bufs=1) as wvp,
            tc.tile_pool(name="ttp", bufs=ND) as ttp,
            tc.tile_pool(name="ptp", bufs=NK) as ptp,
            tc.tile_pool(name="dgp", bufs=2) as dgp,
            tc.tile_pool(name="sgp", bufs=2) as sgp,
            tc.tile_pool(name="pxp", bufs=3) as pxp,
            tc.tile_pool(name="pxtp", bufs=NQT) as pxtp,
            tc.tile_pool(name="op", bufs=3) as op,
        ):
            fb = cst.tile([P, 1], F32)
            # layout matches HBM exactly: [half, d, c, q, i] so every input
            # DMA is fully contiguous on both sides (line-rate transfers)
            xk = xp.tile([P, 2, ND, 4, 2, P], BF16)
            x2 = x2p.tile([P, NK * (D + 2)], BF16)
            wvsb = wvp.tile([P, ND * D], BF16)
            tts = []

            with (
                tc.tile_pool(name="mp", bufs=1) as mp,
                tc.tile_pool(name="psj", bufs=ND, space="PSUM") as psj,
                tc.tile_pool(name="pss", bufs=2, space="PSUM") as pss,
            ):
                # ---- HAM warmup: dummy matmuls on a memset tile keep the
                # PE busy during the initial DMA wait so the clock gate
                # flips to 2.4 GHz before real work arrives
                wmov = wup.tile([P, P], BF16)
                wps = psj.tile([P, 512], F32, tag="ps")
                nc.vector.memset(wmov[:], 0)
                for _ in range(N_WARMUP):
                    nc.tensor.matmul(wps[:, 0:P], wmov[:], wmov[:],
                                     start=True, stop=True)

                # ---- input DMA schedule: TT-critical chunks first, in the
                # order the d-outer TT loop consumes them
                msb = mp.tile([P, ND, ND, P], BF16)
                # d0 split per et-triple so the first TT matmul (et=0)
                # waits for only half the m-d0 chunk
                nc.scalar.dma_start(out=msb[:, 0, 0:3], in_=m_d[:, 0:3 * P])
                nc.scalar.dma_start(out=msb[:, 0, 3:6], in_=m_d[:, 3 * P:D])
                for dd in range(1, ND):
                    nc.scalar.dma_start(
                        out=msb[:, dd], in_=m_d[:, D * dd:D * (dd + 1)])
                nc.scalar.dma_start(out=fb[:], in_=fb_d[:, :])
                # even-j (query) tiles of d-block dd, split per c-pair so
                # the qi=0 sweep starts after a 128KB chunk and the qi=1
                # half streams behind it
                for dd in range(ND):
                    nc.sync.dma_start(
                        out=xk[:, 0, dd, 0:2],
                        in_=xk_d[:, 1024 * dd:1024 * dd + 512])
                for dd in range(ND):
                    nc.sync.dma_start(
                        out=xk[:, 0, dd, 2:4],
                        in_=xk_d[:, 1024 * dd + 512:1024 * (dd + 1)])
                for dd in range(ND):
                    nc.sync.dma_start(
                        out=xk[:, 1, dd],
                        in_=xk_d[:, HLF + 1024 * dd:HLF + 1024 * (dd + 1)])
                w2 = NK * (D + 2) // 2
                nc.sync.dma_start(out=x2[:, 0:w2], in_=x2_d[:, 0:w2])

                # ---- TT projection: TT[d', i] = sum_d M[d, d'] xq^T[d, i].
                # Moving operand = even-j columns of xk. d-outer / et-inner
                # tracks the d-major DMA arrival order; all 6 et accumulate
                # at once so each qi pass makes one slow sweep over the
                # arriving d-chunks (PE never outruns the DMA stream).
                for et in range(ND):
                    tt = ttp.tile([P, H], BF16)
                    tts.append(tt)
                for qi in range(2):
                    accs = {}
                    for et in range(ND):
                        accs[et] = psj.tile([P, 512], F32, tag="ps",
                                            name=f"acc{qi}_{et}")
                    for dd in range(ND):
                        for et in range(ND):
                            nc.tensor.matmul(
                                accs[et][:],
                                msb[:, dd, et, :],
                                xk[:, 0, dd, 2 * qi:2 * qi + 2, :, :],
                                start=(dd == 0), stop=(dd == ND - 1),
                            )
                    for et in range(ND):
                        nc.vector.tensor_copy(
                            tts[et][:, 512 * qi:512 * (qi + 1)], accs[et][:])
                    if qi == 0:
                        # gate the non-TT-critical input DMAs behind the
                        # qi=0 sweep: the 1-column writes below create WAW
                        # dependencies so x2's second half and wv don't
                        # steal SDMA bandwidth from the TT-critical stream
                        nc.vector.tensor_copy(x2[:, w2:w2 + 1],
                                              tts[ND - 1][:, 0:1])
                        nc.vector.tensor_copy(wvsb[:, 0:1],
                                              tts[ND - 1][:, 0:1])
                        nc.scalar.dma_start(out=x2[:, w2:2 * w2],
                                            in_=x2_d[:, w2:2 * w2])
                        nc.scalar.dma_start(out=wvsb[:], in_=wvT_d[:, :])

                # scores share this PSUM scope (psj 6 + pss 2 banks),
                # avoiding a pool-close barrier between TT and scores
                pts = _scores_phase(nc, tc, fb, xk, tts, ptp, dgp, pss)

            # ---- Px = P @ [x | 1 1] per query tile k, then out = Pxn @ wv^T.
            # The ones columns give the softmax denominator in px[:, 768];
            # the divide lands on the bf16 Pxn copy; the d<->i transpose for
            # the final contraction runs on the DMA xbar, not the PE.
            with (
                tc.tile_pool(name="ppx", bufs=2, space="PSUM") as ppx,
                tc.tile_pool(name="pfa", bufs=2, space="PSUM") as pfa,
                tc.tile_pool(name="pfb", bufs=2, space="PSUM") as pfb,
            ):
                def px_chain(k):
                    px = ppx.tile([P, D + 2], F32, tag="px")
                    js = [15] + list(range(2 * k + 1))
                    for idx, j in enumerate(js):
                        k0 = 0 if j == 15 else _k0(j)
                        koff = (k - k0) * P
                        for e0, ew in ((0, 512), (512, D + 2 - 512)):
                            nc.tensor.matmul(
                                px[:, e0:e0 + ew],
                                pts[j][:, koff:koff + P],
                                x2[:, (D + 2) * j + e0:(D + 2) * j + e0 + ew],
                                start=(idx == 0), stop=(idx == len(js) - 1),
                            )
                    rcp = sgp.tile([P, 1], F32, tag="rcp")
                    nc.vector.reciprocal(rcp[:], px[:, D:D + 1])
                    pxn = pxp.tile([P, D], BF16, tag="pxn")
                    nc.vector.tensor_scalar_mul(pxn[:], px[:, :D], rcp[:])
                    pxt = pxtp.tile([P, ND, P], BF16, tag="pxt")
                    # issue on the Scalar queue: Sync is busy with out-DMAs
                    # and semaphores here, which delays the transpose
                    nc.scalar.dma_start_transpose(out=pxt[:], in_=pxn[:])
                    return pxt

                def fin_chain(k, pxt):
                    # two double-buffered PSUM halves so the next fin chain
                    # never waits on this one's drain copies
                    fa = pfa.tile([P, 512], F32, tag="fa")
                    fb2 = pfb.tile([P, 256], F32, tag="fb")
                    for di in range(ND):
                        for dst, e0, ew in ((fa, 0, 512), (fb2, 512, 256)):
                            nc.tensor.matmul(
                                dst[:, :ew],
                                pxt[:, di, :],
                                wvsb[:, D * di + e0:D * di + e0 + ew],
                                start=(di == 0), stop=(di == ND - 1),
                            )
                    if k == 0:
                        # last chain: drain + DMA in independent halves on
                        # separate engines to shorten the kernel tail
                        o1 = op.tile([P, 512], BF16, tag="o1")
                        o2 = op.tile([P, 256], BF16, tag="o2")
                        nc.vector.tensor_copy(o1[:], fa[:])
                        nc.sync.dma_start(out=out_d[0:P, 0:512], in_=o1[:])
                        nc.scalar.activation(
                            o2[:], fb2[:], mybir.ActivationFunctionType.Copy)
                        nc.scalar.dma_start(out=out_d[0:P, 512:D], in_=o2[:])
                    else:
                        o = op.tile([P, D], BF16, tag="o")
                        if k % 2:
                            # drain on vector, trigger on scalar
                            nc.vector.tensor_copy(o[:, 0:512], fa[:])
                            nc.vector.tensor_copy(o[:, 512:D], fb2[:])
                            deng = nc.scalar
                        else:
                            # drain on scalar (ACT copy), trigger on sync —
                            # splits the drain load across two engines so
                            # PSUM recycling never waits on a busy DVE
                            nc.scalar.activation(
                                o[:, 0:512], fa[:],
                                mybir.ActivationFunctionType.Copy)
                            nc.scalar.activation(
                                o[:, 512:D], fb2[:],
                                mybir.ActivationFunctionType.Copy)
                            deng = nc.sync
                        deng.dma_start(out=out_d[k * P:(k + 1) * P, :], in_=o[:])

                # all px chains first (divides + xbar transposes trail on
                # Vector/DMA), then all fin chains - by fin time every pxt
                # is ready, so the PE never waits on the transpose latency
                pxts = {k: px_chain(k) for k in range(NQT - 1, -1, -1)}
                for k in range(NQT - 1, -1, -1):
                    fin_chain(k, pxts.pop(k))

    nc.compile()
    return nc


def _get_nc():
    if "nc" not in _cached:
        _cached["nc"] = _build_nc()
    return _cached["nc"]


def kernel(x, w_q, w_k, w_v):
    global last_results
    x = np.ascontiguousarray(np.asarray(x, dtype=np.float32))
    w_q = np.asarray(w_q, dtype=np.float32)
    w_k = np.asarray(w_k, dtype=np.float32)
    w_v = np.asarray(w_v, dtype=np.float32)

    def pack_w(w):
        # [768, 768] -> [128, 6*768] with d-blocks along columns
        return np.ascontiguousarray(
            w.reshape(ND, P, D).transpose(1, 0, 2).reshape(P, ND * D)).astype(BF)

    # m d-major: col = 768*d + 128*et + c2 -> M[128d+p, 128et+c2]
    m = np.ascontiguousarray(
        (w_q.T @ w_k).reshape(ND, P, ND, P).transpose(1, 0, 2, 3).reshape(P, ND * D)
    ).astype(BF)
    wvT = pack_w(np.ascontiguousarray(w_v.T))

    nc = _get_nc()
    in_maps = []
    for core in range(8):
        b, h = core // 2, core % 2
        r = P * h
        rot = np.concatenate([x[b, r:], x[b, :r]], axis=0)
        # tiles [j][d, p, i] = rot[128j+i, 128d+p]
        tl = rot.reshape(NK, P, ND, P).transpose(0, 2, 3, 1)   # [j, d, p, i]
        xk_half = []
        for js in ((0, 2), (1, 3)):
            # [d, c, q, p, i] -> [p, d, c, q, i]
            blk = np.stack(
                [np.stack([tl[4 * c + jq] for jq in js], axis=1)
                 for c in range(4)], axis=1)                    # [d, c, q, p, i]
            xk_half.append(blk.transpose(3, 0, 1, 2, 4).reshape(P, ND * S // 2))
        xk = np.ascontiguousarray(np.concatenate(xk_half, axis=1)).astype(BF)
        x2 = np.ascontiguousarray(
            np.concatenate([rot, np.ones((S, 2), np.float32)], axis=1)
            .reshape(NK, P, D + 2).transpose(1, 0, 2).reshape(P, NK * (D + 2))
        ).astype(BF)
        in_maps.append({
            "m": m,
            "xk": xk,
            "x2": x2,
            "wvT": wvT,
            "fbias": np.full((P, 1), 0.0 if h == 1 else NEG, np.float32),
        })

    trace = bool(int(os.environ.get("KERNEL_TRACE", "0")))
    res = run_bass_kernel_spmd(nc, in_maps, core_ids=list(range(8)), trace=trace)
    last_results = res

    out = np.empty((B, S, D), np.float32)
    for core in range(8):
        b, h = core // 2, core % 2
        o = np.asarray(res.results[core]["out"]).astype(np.float32)
        out[b].reshape(NK, P, D)[h::2] = o.reshape(NQT, P, D)
    return out


# revision 29
# speedup vs baseline: 1.0215x; 1.0215x over previous
"""Causal single-head attention (B=4, S=2048, D=768) on 8 TRN2 NeuronCores.

Sharding: core (b, h) = batch b, query-interleave h. Each core computes the
attention output for query tiles {2k+h : k=0..7} (128 rows each) of one
batch. Keys are fed ROTATED by 128*h so every core sees the identical score
structure: query tile k sits at rotated row 256k and attends rotated key
tiles 0..2k (tile 2k triangular) plus the wrap tile 15, which holds the
original first 128 keys for h=1 and is killed via the exp bias for h=0.

QK merge: scores = x (W_q^T W_k) x^T, with M = W_q^T W_k computed on the
host. The kernel computes TT = M^T xq^T (one projection instead of Q and K)
and uses the resident x tiles directly as the score stationaries. The TT
moving operand is read straight out of the xk tile (queries sit at even
rotated tiles), so no separate xq tensor is shipped.

xk layout [P, half(2), d(6), c(4), q(2), 128] matches HBM exactly (all DMAs
contiguous at line rate): even-j (query) tiles in half 0, d-major, shipped
first, so the TT phase (d-outer, et-inner) starts after ~160KB of DMA and
tracks arrival order. Non-TT-critical inputs (x2 second half, wv) are gated
behind the TT qi=0 sweep via WAW-dependency writes so they cannot steal
SDMA bandwidth from the critical stream.

All matmul inputs are bf16; accumulation stays f32 in PSUM; the output is
shipped bf16 and upcast on the host. Softmax denominators come from two
ones-columns appended to V (cols 768..769). Dummy matmuls at program start
keep the PE busy through the initial DMA wait so the HAM clock gate flips
to 2.4 GHz once, early, and never re-throttles.
"""

import os
import numpy as np
import ml_dtypes

import concourse.bass as bass
import concourse.mybir as mybir
import concourse.tile as tile
from concourse import bacc
from concourse.bass_utils import run_bass_kernel_spmd

B, S, D = 4, 2048, 768
P = 128
ND = D // P          # 6 contraction tiles
NQT = 8              # query tiles per core (128 rows each)
H = NQT * P          # 1024 query rows per core
NK = S // P          # 16 key tiles
SCALE = 1.0 / float(np.sqrt(D))
NEG = -10000.0
F32 = mybir.dt.float32
BF16 = mybir.dt.bfloat16
BF = ml_dtypes.bfloat16
N_WARMUP = 30        # dummy matmuls to warm the HAM clock gate (N=128 each)

_cached = {}
last_results = None

# jm position of tile j within its c-block: even j's first
_JMPOS = {0: 0, 2: 1, 1: 2, 3: 3}


def _k0(j):
    # first query tile whose score group includes key tile j (j < 15)
    return (j + 1) // 2


def _scores_phase(nc, tc, fb, xk, tts, ptp, dgp, pss):
    def xst(j):
        # stationary for key tile j: [128 d-rows, 128 key cols]
        return lambda dp: xk[:, j & 1, dp, j // 4, (j % 4) // 2, :]

    pts = {}
    for j in [15] + list(range(15)):
        k0 = 0 if j == 15 else _k0(j)
        ncol = (NQT - k0) * P
        pt = ptp.tile([P, H], BF16, name=f"pt{j}", tag="pt")
        pts[j] = pt
        st_j = xst(j)
        for qoff in range(0, ncol, 512):
            qw = min(512, ncol - qoff)
            st = pss.tile([P, 512], F32, tag="st")
            for dp in range(ND):
                nc.tensor.matmul(
                    st[:, :qw],
                    st_j(dp),
                    tts[dp][:, k0 * P + qoff:k0 * P + qoff + qw],
                    start=(dp == 0), stop=(dp == ND - 1),
                )
            if j == 15:
                nc.scalar.activation(
                    pt[:, qoff:qoff + qw], st[:, :qw],
                    mybir.ActivationFunctionType.Exp,
                    bias=fb[:, 0:1], scale=SCALE,
                )
            elif qoff == 0 and j % 2 == 0:
                # even j: leading tile is the diagonal triangle; odd j's
                # leading tile is already strictly below the diagonal
                mw = min(256, ncol)
                dg = dgp.tile([P, 256], F32, tag="dg")
                nc.vector.tensor_copy(dg[:, :mw], st[:, 0:mw])
                # keep where 256*k0 + 256*kk + f - (128*j + p) >= 0
                nc.gpsimd.affine_select(
                    out=dg[:, :mw], in_=dg[:, :mw],
                    compare_op=mybir.AluOpType.is_ge,
                    fill=NEG, base=256 * k0 - P * j,
                    pattern=([[256, 2], [1, P]] if mw == 256 else [[1, P]]),
                    channel_multiplier=-1,
                )
                nc.scalar.activation(
                    pt[:, 0:mw], dg[:, :mw],
                    mybir.ActivationFunctionType.Exp,
                    bias=0.0, scale=SCALE,
                )
                if qw > mw:
                    nc.scalar.activation(
                        pt[:, mw:qw], st[:, mw:qw],
                        mybir.ActivationFunctionType.Exp,
                        bias=0.0, scale=SCALE,
                    )
            else:
                nc.scalar.activation(
                    pt[:, qoff:qoff + qw], st[:, :qw],
                    mybir.ActivationFunctionType.Exp,
                    bias=0.0, scale=SCALE,
                )
    return pts


def _build_nc():
    nc = bacc.Bacc("TRN2", target_bir_lowering=False)

    # m d-major: col = 768*d + 128*et + c2  ->  M[128d+p, 128et+c2]
    m_d = nc.dram_tensor("m", [P, ND * D], BF16, kind="ExternalInput")
    # x keys: even-j half then odd-j half, d-major inside each:
    # even: col = 1024*d + 256*c + 128*q + i  (q=0,1 -> j = 4c, 4c+2)
    # odd:  col = 6144 + 1024*d + 256*c + 128*q + i  (j = 4c+1, 4c+3)
    xk_d = nc.dram_tensor("xk", [P, ND * S], BF16, kind="ExternalInput")
    # x keys again in [j, d] row layout + two ones columns, per-tile blocks
    x2_d = nc.dram_tensor("x2", [P, NK * (D + 2)], BF16, kind="ExternalInput")
    wvT_d = nc.dram_tensor("wvT", [P, ND * D], BF16, kind="ExternalInput")
    fb_d = nc.dram_tensor("fbias", [P, 1], F32, kind="ExternalInput")
    out_d = nc.dram_tensor("out", [H, D], BF16, kind="ExternalOutput")

    HLF = ND * S // 2    # 6144: column offset of the odd-j half in xk_d

    with tile.TileContext(nc) as tc:
        with (
            tc.tile_pool(name="cst", bufs=1) as cst,
            tc.tile_pool(name="wup", bufs=1) as wup,
            tc.tile_pool(name="xp", bufs=1) as xp,
            tc.tile_pool(name="x2p", bufs=1) as x2p,
            tc.tile_pool(name="wvp", bufs=1) as wvp,
            tc.tile_pool(name="ttp", bufs=ND) as ttp,
            tc.tile_pool(name="ptp", bufs=NK) as ptp,
            tc.tile_pool(name="dgp", bufs=2) as dgp,
            tc.tile_pool(name="sgp", bufs=2) as sgp,
            tc.tile_pool(name="pxp", bufs=3) as pxp,
            tc.tile_pool(name="pxtp", bufs=NQT) as pxtp,
            tc.tile_pool(name="op", bufs=3) as op,
        ):
            fb = cst.tile([P, 1], F32)
            # layout matches HBM exactly: [half, d, c, q, i] so every input
            # DMA is fully contiguous on both sides (line-rate transfers)
            xk = xp.tile([P, 2, ND, 4, 2, P], BF16)
            x2 = x2p.tile([P, NK * (D + 2)], BF16)
            wvsb = wvp.tile([P, ND * D], BF16)
            tts = []

            with (
                tc.tile_pool(name="mp", bufs=1) as mp,
                tc.tile_pool(name="psj", bufs=ND, space="PSUM") as psj,
                tc.tile_pool(name="pss", bufs=2, space="PSUM") as pss,
            ):
                # ---- HAM warmup: dummy matmuls on a memset tile keep the
                # PE busy during the initial DMA wait so the clock gate
                # flips to 2.4 GHz before real work arrives
                wmov = wup.tile([P, P], BF16)
                wps = psj.tile([P, 512], F32, tag="ps")
                nc.vector.memset(wmov[:], 0)
                for _ in range(N_WARMUP):
                    nc.tensor.matmul(wps[:, 0:P], wmov[:], wmov[:],
                                     start=True, stop=True)

                # ---- input DMA schedule: TT-critical chunks first, in the
                # order the d-outer TT loop consumes them
                msb = mp.tile([P, ND, ND, P], BF16)
                # d0 split per et-triple so the first TT matmul (et=0)
                # waits for only half the m-d0 chunk
                nc.scalar.dma_start(out=msb[:, 0, 0:3], in_=m_d[:, 0:3 * P])
                nc.scalar.dma_start(out=msb[:, 0, 3:6], in_=m_d[:, 3 * P:D])
                for dd in range(1, ND):
                    nc.scalar.dma_start(
                        out=msb[:, dd], in_=m_d[:, D * dd:D * (dd + 1)])
                nc.scalar.dma_start(out=fb[:], in_=fb_d[:, :])
                # even-j (query) tiles of d-block dd, split per c-pair so
                # the qi=0 sweep starts after a small chunk and the qi=1
                # half streams behind it; d0's pair is split once more so
                # the very first matmul's sem lands as early as possible
                nc.sync.dma_start(out=xk[:, 0, 0, 0:1], in_=xk_d[:, 0:256])
                nc.sync.dma_start(out=xk[:, 0, 0, 1:2], in_=xk_d[:, 256:512])
                for dd in range(1, ND):
                    nc.sync.dma_start(
                        out=xk[:, 0, dd, 0:2],
                        in_=xk_d[:, 1024 * dd:1024 * dd + 512])
                for dd in range(ND):
                    nc.sync.dma_start(
                        out=xk[:, 0, dd, 2:4],
                        in_=xk_d[:, 1024 * dd + 512:1024 * (dd + 1)])
                for dd in range(ND):
                    nc.sync.dma_start(
                        out=xk[:, 1, dd],
                        in_=xk_d[:, HLF + 1024 * dd:HLF + 1024 * (dd + 1)])
                w2 = NK * (D + 2) // 2
                nc.sync.dma_start(out=x2[:, 0:w2], in_=x2_d[:, 0:w2])

                # ---- TT projection: TT[d', i] = sum_d M[d, d'] xq^T[d, i].
                # Moving operand = even-j columns of xk. d-outer / et-inner
                # tracks the d-major DMA arrival order; all 6 et accumulate
                # at once so each qi pass makes one slow sweep over the
                # arriving d-chunks (PE never outruns the DMA stream).
                for et in range(ND):
                    tt = ttp.tile([P, H], BF16)
                    tts.append(tt)
                for qi in range(2):
                    accs = {}
                    for et in range(ND):
                        accs[et] = psj.tile([P, 512], F32, tag="ps",
                                            name=f"acc{qi}_{et}")
                    for dd in range(ND):
                        for et in range(ND):
                            nc.tensor.matmul(
                                accs[et][:],
                                msb[:, dd, et, :],
                                xk[:, 0, dd, 2 * qi:2 * qi + 2, :, :],
                                start=(dd == 0), stop=(dd == ND - 1),
                            )
                    for et in range(ND):
                        nc.vector.tensor_copy(
                            tts[et][:, 512 * qi:512 * (qi + 1)], accs[et][:])
                    if qi == 0:
                        # gate the non-TT-critical input DMAs behind the
                        # qi=0 sweep: the 1-column writes below create WAW
                        # dependencies so x2's second half and wv don't
                        # steal SDMA bandwidth from the TT-critical stream
                        nc.vector.tensor_copy(x2[:, w2:w2 + 1],
                                              tts[ND - 1][:, 0:1])
                        nc.vector.tensor_copy(wvsb[:, 0:1],
                                              tts[ND - 1][:, 0:1])
                        nc.scalar.dma_start(out=x2[:, w2:2 * w2],
                                            in_=x2_d[:, w2:2 * w2])
                        nc.scalar.dma_start(out=wvsb[:], in_=wvT_d[:, :])

                # scores share this PSUM scope (psj 6 + pss 2 banks),
                # avoiding a pool-close barrier between TT and scores
                pts = _scores_phase(nc, tc, fb, xk, tts, ptp, dgp, pss)

            # ---- Px = P @ [x | 1 1] per query tile k, then out = Pxn @ wv^T.
            # The ones columns give the softmax denominator in px[:, 768];
            # the divide lands on the bf16 Pxn copy; the d<->i transpose for
            # the final contraction runs on the DMA xbar, not the PE.
            with (
                tc.tile_pool(name="ppx", bufs=2, space="PSUM") as ppx,
                tc.tile_pool(name="pfa", bufs=2, space="PSUM") as pfa,
                tc.tile_pool(name="pfb", bufs=2, space="PSUM") as pfb,
            ):
                def px_chain(k):
                    px = ppx.tile([P, D + 2], F32, tag="px")
                    js = [15] + list(range(2 * k + 1))
                    for idx, j in enumerate(js):
                        k0 = 0 if j == 15 else _k0(j)
                        koff = (k - k0) * P
                        for e0, ew in ((0, 512), (512, D + 2 - 512)):
                            nc.tensor.matmul(
                                px[:, e0:e0 + ew],
                                pts[j][:, koff:koff + P],
                                x2[:, (D + 2) * j + e0:(D + 2) * j + e0 + ew],
                                start=(idx == 0), stop=(idx == len(js) - 1),
                            )
                    rcp = sgp.tile([P, 1], F32, tag="rcp")
                    nc.vector.reciprocal(rcp[:], px[:, D:D + 1])
                    pxn = pxp.tile([P, D], BF16, tag="pxn")
                    nc.vector.tensor_scalar_mul(pxn[:], px[:, :D], rcp[:])
                    pxt = pxtp.tile([P, ND, P], BF16, tag="pxt")
                    # issue on the Scalar queue: Sync is busy with out-DMAs
                    # and semaphores here, which delays the transpose
                    nc.scalar.dma_start_transpose(out=pxt[:], in_=pxn[:])
                    return pxt

                def fin_chain(k, pxt):
                    # two double-buffered PSUM halves so the next fin chain
                    # never waits on this one's drain copies
                    fa = pfa.tile([P, 512], F32, tag="fa")
                    fb2 = pfb.tile([P, 256], F32, tag="fb")
                    for di in range(ND):
                        for dst, e0, ew in ((fa, 0, 512), (fb2, 512, 256)):
                            nc.tensor.matmul(
                                dst[:, :ew],
                                pxt[:, di, :],
                                wvsb[:, D * di + e0:D * di + e0 + ew],
                                start=(di == 0), stop=(di == ND - 1),
                            )
                    if k == 0:
                        # last chain: drain + DMA in independent halves on
                        # separate engines to shorten the kernel tail
                        o1 = op.tile([P, 512], BF16, tag="o1")
                        o2 = op.tile([P, 256], BF16, tag="o2")
                        nc.vector.tensor_copy(o1[:], fa[:])
                        nc.sync.dma_start(out=out_d[0:P, 0:512], in_=o1[:])
                        nc.scalar.activation(
                            o2[:], fb2[:], mybir.ActivationFunctionType.Copy)
                        nc.scalar.dma_start(out=out_d[0:P, 512:D], in_=o2[:])
                    else:
                        o = op.tile([P, D], BF16, tag="o")
                        if k % 2:
                            # drain on vector, trigger on scalar
                            nc.vector.tensor_copy(o[:, 0:512], fa[:])
                            nc.vector.tensor_copy(o[:, 512:D], fb2[:])
                            deng = nc.scalar
                        else:
                            # drain on scalar (ACT copy), trigger on sync —
                            # splits the drain load across two engines so
                            # PSUM recycling never waits on a busy DVE
                            nc.scalar.activation(
                                o[:, 0:512], fa[:],
                                mybir.ActivationFunctionType.Copy)
                            nc.scalar.activation(
                                o[:, 512:D], fb2[:],
                                mybir.ActivationFunctionType.Copy)
                            deng = nc.sync
                        deng.dma_start(out=out_d[k * P:(k + 1) * P, :], in_=o[:])

                # all px chains first (divides + xbar transposes trail on
                # Vector/DMA), then all fin chains - by fin time every pxt
                # is ready, so the PE never waits on the transpose latency
                pxts = {k: px_chain(k) for k in range(NQT - 1, -1, -1)}
                for k in range(NQT - 1, -1, -1):
                    fin_chain(k, pxts.pop(k))

    nc.compile()
    return nc


def _get_nc():
    if "nc" not in _cached:
        _cached["nc"] = _build_nc()
    return _cached["nc"]


def kernel(x, w_q, w_k, w_v):
    global last_results
    x = np.ascontiguousarray(np.asarray(x, dtype=np.float32))
    w_q = np.asarray(w_q, dtype=np.float32)
    w_k = np.asarray(w_k, dtype=np.float32)
    w_v = np.asarray(w_v, dtype=np.float32)

    def pack_w(w):
        # [768, 768] -> [128, 6*768] with d-blocks along columns
        return np.ascontiguousarray(
            w.reshape(ND, P, D).transpose(1, 0, 2).reshape(P, ND * D)).astype(BF)

    # m d-major: col = 768*d + 128*et + c2 -> M[128d+p, 128et+c2]
    m = np.ascontiguousarray(
        (w_q.T @ w_k).reshape(ND, P, ND, P).transpose(1, 0, 2, 3).reshape(P, ND * D)
    ).astype(BF)
    wvT = pack_w(np.ascontiguousarray(w_v.T))

    nc = _get_nc()
    in_maps = []
    for core in range(8):
        b, h = core // 2, core % 2
        r = P * h
        rot = np.concatenate([x[b, r:], x[b, :r]], axis=0)
        # tiles [j][d, p, i] = rot[128j+i, 128d+p]
        tl = rot.reshape(NK, P, ND, P).transpose(0, 2, 3, 1)   # [j, d, p, i]
        xk_half = []
        for js in ((0, 2), (1, 3)):
            # [d, c, q, p, i] -> [p, d, c, q, i]
            blk = np.stack(
                [np.stack([tl[4 * c + jq] for jq in js], axis=1)
                 for c in range(4)], axis=1)                    # [d, c, q, p, i]
            xk_half.append(blk.transpose(3, 0, 1, 2, 4).reshape(P, ND * S // 2))
        xk = np.ascontiguousarray(np.concatenate(xk_half, axis=1)).astype(BF)
        x2 = np.ascontiguousarray(
            np.concatenate([rot, np.ones((S, 2), np.float32)], axis=1)
            .reshape(NK, P, D + 2).transpose(1, 0, 2).reshape(P, NK * (D + 2))
        ).astype(BF)
        in_maps.append({
            "m": m,
            "xk": xk,
            "x2": x2,
            "wvT": wvT,
            "fbias": np.full((P, 1), 0.0 if h == 1 else NEG, np.float32),
        })

    trace = bool(int(os.environ.get("KERNEL_TRACE", "0")))
    res = run_bass_kernel_spmd(nc, in_maps, core_ids=list(range(8)), trace=trace)
    last_results = res

    out = np.empty((B, S, D), np.float32)
    for core in range(8):
        b, h = core // 2, core % 2
        o = np.asarray(res.results[core]["out"]).astype(np.float32)
        out[b].reshape(NK, P, D)[h::2] = o.reshape(NQT, P, D)
    return out
